# revision 14
# baseline (speedup 1.0000x reference)
"""MoE grouped linear (gmm) kernel for 8 Trainium2 NeuronCores.

Strategy (expert parallel, mirrors the shard_map-over-gmm_sharded source):
  - Tokens arrive pre-sorted by expert; group_sizes[e] tokens belong to
    expert e. Core e gets weight[e] plus expert e's token slice, padded to
    MAXG rows so all 8 cores run one SPMD program. The "all-to-all" routing
    is host-side slicing, since kernel() sees the full inputs.
  - Per core we compute y_e^T = W_e^T @ X_e^T (out^T orientation): the
    weight tiles are the PE's stationary operand in natural [K, O] layout
    and X^T (prepared host-side) streams as the moving operand.
  - fp32 inputs are DMA'd untouched into resident SBUF tiles; the PE reads
    the high half of each fp32 word as bf16 through a bitcast + stride-2
    access pattern (truncation toward zero). The mean truncation shrink is
    measured host-side and compensated via the ScalarE evacuation scale;
    the per-partition bias is fused into the same instruction. PSUM
    accumulates in fp32.
Host then unpads/concatenates per-expert outputs back to [T, Out] fp32.
"""

import numpy as np

import concourse.bass as bass
from concourse import bacc
import concourse.mybir as mybir
import concourse.tile as tile
from concourse.bass_utils import run_bass_kernel_spmd

N_CORES = 8
P = 128

_BUILD_CACHE: dict = {}


def _t_chunks(maxg: int) -> list[tuple[int, int]]:
    """Split the token free-dim into PSUM-bank-sized (<=512) chunks."""
    n = (maxg + 511) // 512
    base = ((maxg // n + P - 1) // P) * P
    chunks = []
    off = 0
    while off < maxg:
        sz = min(base, maxg - off)
        chunks.append((off, sz))
        off += sz
    return chunks


def _build_program(maxg: int, n_in: int, n_out: int):
    kb = n_in // P   # contraction blocks
    ob = n_out // P  # output-row blocks
    f32 = mybir.dt.float32
    bf16 = mybir.dt.bfloat16

    nc = bacc.Bacc(
        "TRN2", target_bir_lowering=False, debug=False, num_devices=N_CORES
    )
    xt = nc.dram_tensor("xt", [n_in, maxg], f32, kind="ExternalInput")
    # W pre-tiled host-side: [ob, P(partition of k-block), kb, P(o)] so each
    # o-slab DMA is fully contiguous per partition (8 KiB segments).
    w = nc.dram_tensor("w", [ob, P, kb, P], f32, kind="ExternalInput")
    bias = nc.dram_tensor("bias", [P, ob], f32, kind="ExternalInput")
    sc = nc.dram_tensor("sc", [P, 1], f32, kind="ExternalInput")
    yt = nc.dram_tensor("yt", [n_out, maxg], f32, kind="ExternalOutput")

    chunks = _t_chunks(maxg)

    with tile.TileContext(nc) as tc:
        with (
            tc.tile_pool(name="const", bufs=1) as constp,
            tc.tile_pool(name="xtsb", bufs=1) as xtp,
            tc.tile_pool(name="wsb", bufs=4) as wp,
            tc.tile_pool(name="outsb", bufs=4) as outp,
            tc.tile_pool(name="psum", bufs=4, space="PSUM") as psump,
        ):
            bias_sb = constp.tile([P, ob], f32)
            nc.sync.dma_start(bias_sb[:], bias[:])
            sc_sb = constp.tile([P, 1], f32)
            nc.sync.dma_start(sc_sb[:], sc[:])

            # X^T resident in fp32, one tile per k-slab so the PE can start
            # as soon as the first slabs land; bf16 view = high half of word.
            xvs = []
            for k in range(kb):
                xk = xtp.tile([P, maxg], f32, tag=f"x{k}", name=f"xk{k}")
                nc.sync.dma_start(xk[:], xt[k * P : (k + 1) * P, :])
                xvs.append(
                    xk[:].bitcast(bf16).rearrange("p (t two) -> p t two", two=2)
                )

            def load_w(o):
                w_o = wp.tile([P, kb, P], f32, tag="wo", name=f"w{o}")
                nc.sync.dma_start(w_o[:], w[o])
                return w_o

            # Stream W one o-slab (all k, 128 output cols) at a time; the
            # slab's bf16 view is the PE stationary operand. Loop order
            # o -> k -> t so one LDWEIGHTS serves every t-chunk.
            w_pref = {0: load_w(0)}
            for o in range(ob):
                w_o = w_pref.pop(o) if o in w_pref else load_w(o)
                wov = (
                    w_o[:]
                    .bitcast(bf16)
                    .rearrange("p k (o two) -> p k o two", two=2)
                )
                pss = [
                    psump.tile(
                        [P, tsz], f32, tag=f"ps{ti}", name=f"ps{o}_{ti}"
                    )
                    for ti, (t0, tsz) in enumerate(chunks)
                ]
                for k in range(kb):
                    for ti, (t0, tsz) in enumerate(chunks):
                        nc.tensor.matmul(
                            pss[ti][:],
                            wov[:, k, :, 1],
                            xvs[k][:, t0 : t0 + tsz, 1],
                            start=(k == 0),
                            stop=(k == kb - 1),
                        )
                for ti, (t0, tsz) in enumerate(chunks):
                    ot = outp.tile([P, tsz], f32, tag="ot")
                    nc.scalar.activation(
                        ot[:],
                        pss[ti][:],
                        mybir.ActivationFunctionType.Identity,
                        bias=bias_sb[:, o : o + 1],
                        scale=sc_sb[:, 0:1],
                    )
                    nc.sync.dma_start(
                        yt[o * P : (o + 1) * P, t0 : t0 + tsz], ot[:]
                    )
    nc.finalize()
    return nc


def _trunc_ratio(a: np.ndarray) -> float:
    """mean(|trunc_bf16(a)|) / mean(|a|) — the systematic shrink from
    reading only the high 16 bits of each fp32."""
    t = (a.view(np.uint32) & np.uint32(0xFFFF0000)).view(np.float32)
    denom = float(np.abs(a).sum())
    if denom == 0.0:
        return 1.0
    return float(np.abs(t).sum()) / denom


def _prepare(inputs, weight, bias, group_sizes):
    """Build (or reuse) the program and the per-core input maps."""
    inputs = np.ascontiguousarray(np.asarray(inputs, dtype=np.float32))
    weight = np.ascontiguousarray(np.asarray(weight, dtype=np.float32))
    bias = np.ascontiguousarray(np.asarray(bias, dtype=np.float32))
    g = np.asarray(group_sizes).astype(np.int64)

    t_tokens, n_in = inputs.shape
    n_exp, _, n_out = weight.shape
    assert n_exp == N_CORES, f"expected {N_CORES} experts, got {n_exp}"
    offs = np.concatenate([[0], np.cumsum(g)])
    assert offs[-1] == t_tokens, "group_sizes must sum to token count"

    maxg = max(P, int(-(-int(g.max()) // P)) * P)

    key = (maxg, n_in, n_out)
    if key not in _BUILD_CACHE:
        _BUILD_CACHE[key] = _build_program(maxg, n_in, n_out)
    nc = _BUILD_CACHE[key]

    ob = n_out // P
    bias_host = np.ascontiguousarray(bias.reshape(ob, P).T)  # [P, ob]

    # Compensate the mean truncation shrink of both operands.
    scale = 1.0 / (_trunc_ratio(inputs) * _trunc_ratio(weight))
    sc_host = np.full((P, 1), scale, np.float32)

    in_maps = []
    for e in range(n_exp):
        xe = inputs[offs[e] : offs[e + 1]]  # [g_e, n_in]
        xt_e = np.zeros((n_in, maxg), np.float32)
        xt_e[:, : g[e]] = xe.T
        w_e = np.ascontiguousarray(
            weight[e].reshape(kb := n_in // P, P, ob, P).transpose(2, 1, 0, 3)
        )  # [ob, P(k within block), kb, P(o)]
        in_maps.append(
            {"xt": xt_e, "w": w_e, "bias": bias_host, "sc": sc_host}
        )
    return nc, in_maps, g, offs, (t_tokens, n_out)


def kernel(inputs, weight, bias, group_sizes):
    nc, in_maps, g, offs, (t_tokens, n_out) = _prepare(
        inputs, weight, bias, group_sizes
    )
    res = run_bass_kernel_spmd(nc, in_maps, core_ids=list(range(N_CORES)))

    out = np.empty((t_tokens, n_out), np.float32)
    for e in range(N_CORES):
        if g[e] == 0:
            continue
        yt_e = res.results[e]["yt"]  # [n_out, maxg]
        out[offs[e] : offs[e + 1]] = yt_e[:, : g[e]].T
    return out


# revision 15
# speedup vs baseline: 1.0728x; 1.0728x over previous
"""MoE grouped linear (gmm) kernel for 8 Trainium2 NeuronCores.

Strategy (expert parallel, mirrors the shard_map-over-gmm_sharded source):
  - Tokens arrive pre-sorted by expert; group_sizes[e] tokens belong to
    expert e. Core e gets weight[e] plus expert e's token slice, padded to
    MAXG rows so all 8 cores run one SPMD program. The "all-to-all" routing
    is host-side slicing, since kernel() sees the full inputs.
  - Per core we compute y_e^T = W_e^T @ X_e^T (out^T orientation): the
    weight tiles are the PE's stationary operand in natural [K, O] layout
    and X^T (prepared host-side) streams as the moving operand.
  - fp32 inputs are DMA'd untouched into resident SBUF tiles; the PE reads
    the high half of each fp32 word as bf16 through a bitcast + stride-2
    access pattern (truncation toward zero). The mean truncation shrink is
    measured host-side and compensated via the ScalarE evacuation scale;
    the per-partition bias is fused into the same instruction. PSUM
    accumulates in fp32.
Host then unpads/concatenates per-expert outputs back to [T, Out] fp32.
"""

import numpy as np

import concourse.bass as bass
from concourse import bacc
import concourse.mybir as mybir
import concourse.tile as tile
from concourse.bass_utils import run_bass_kernel_spmd

N_CORES = 8
P = 128

_BUILD_CACHE: dict = {}


def _t_chunks(maxg: int) -> list[tuple[int, int]]:
    """Split the token free-dim into PSUM-bank-sized (<=512) chunks."""
    n = (maxg + 511) // 512
    base = ((maxg // n + P - 1) // P) * P
    chunks = []
    off = 0
    while off < maxg:
        sz = min(base, maxg - off)
        chunks.append((off, sz))
        off += sz
    return chunks


def _build_program(maxg: int, n_in: int, n_out: int):
    kb = n_in // P   # contraction blocks
    ob = n_out // P  # output-row blocks
    f32 = mybir.dt.float32
    bf16 = mybir.dt.bfloat16

    nc = bacc.Bacc(
        "TRN2", target_bir_lowering=False, debug=False, num_devices=N_CORES
    )
    xt = nc.dram_tensor("xt", [n_in, maxg], f32, kind="ExternalInput")
    # W pre-tiled host-side: [ob, P(partition of k-block), kb, P(o)] so each
    # o-slab DMA is fully contiguous per partition (8 KiB segments).
    w = nc.dram_tensor("w", [ob, P, kb, P], f32, kind="ExternalInput")
    bias = nc.dram_tensor("bias", [P, ob], f32, kind="ExternalInput")
    sc = nc.dram_tensor("sc", [P, 1], f32, kind="ExternalInput")
    yt = nc.dram_tensor("yt", [n_out, maxg], f32, kind="ExternalOutput")

    chunks = _t_chunks(maxg)

    GRP = 4  # o-blocks processed concurrently (GRP * len(chunks) PSUM banks)
    XQ = 4   # k-slabs per X quarter-tile

    with tile.TileContext(nc) as tc:
        with (
            tc.tile_pool(name="const", bufs=1) as constp,
            tc.tile_pool(name="xtsb", bufs=1) as xtp,
            tc.tile_pool(name="wsb", bufs=2 * GRP) as wp,
            tc.tile_pool(name="outsb", bufs=2 * GRP) as outp,
            tc.tile_pool(name="psum", bufs=1, space="PSUM") as psump,
        ):
            bias_sb = constp.tile([P, ob], f32)
            nc.sync.dma_start(bias_sb[:], bias[:])
            sc_sb = constp.tile([P, 1], f32)
            nc.sync.dma_start(sc_sb[:], sc[:])

            def load_w(o):
                w_o = wp.tile([P, kb, P], f32, tag="wo", name=f"w{o}")
                nc.sync.dma_start(w_o[:], w[o])
                return w_o

            def load_xq(q):
                xq = xtp.tile([P, XQ, maxg], f32, tag=f"xq{q}", name=f"xq{q}")
                nc.sync.dma_start(
                    xq[:],
                    xt[q * XQ * P : (q + 1) * XQ * P, :].rearrange(
                        "(k p) t -> p k t", p=P
                    ),
                )
                return xq[:].bitcast(bf16).rearrange(
                    "p k (t two) -> p k t two", two=2
                )

            # Interleave the prologue DMAs: the single HW ring delivers in
            # order, and group 0 needs w0..w3 plus all of X before its end.
            w_pref = {}
            xqs = [None] * (kb // XQ)
            for kind, i in (
                ("w", 0), ("x", 0), ("w", 1), ("x", 1),
                ("w", 2), ("w", 3), ("x", 2), ("x", 3),
            ):
                if kind == "w":
                    w_pref[i] = load_w(i)
                else:
                    xqs[i] = load_xq(i)

            # Process o-blocks in groups of GRP: each X-slab arrival feeds
            # GRP o-columns of PE work, and one LDWEIGHTS per (k, o) serves
            # every t-chunk.
            for og in range(0, ob, GRP):
                group = list(range(og, min(og + GRP, ob)))
                wovs = []
                for o in group:
                    w_o = w_pref.pop(o) if o in w_pref else load_w(o)
                    wovs.append(
                        w_o[:]
                        .bitcast(bf16)
                        .rearrange("p k (o two) -> p k o two", two=2)
                    )
                pss = {
                    (oi, ti): psump.tile(
                        [P, tsz],
                        f32,
                        tag=f"ps{oi}_{ti}",
                        name=f"ps{og + oi}_{ti}",
                    )
                    for oi in range(len(group))
                    for ti, (t0, tsz) in enumerate(chunks)
                }
                for k in range(kb):
                    xvk = xqs[k // XQ]
                    for oi in range(len(group)):
                        for ti, (t0, tsz) in enumerate(chunks):
                            nc.tensor.matmul(
                                pss[oi, ti][:],
                                wovs[oi][:, k, :, 1],
                                xvk[:, k % XQ, t0 : t0 + tsz, 1],
                                start=(k == 0),
                                stop=(k == kb - 1),
                            )
                for oi, o in enumerate(group):
                    for ti, (t0, tsz) in enumerate(chunks):
                        ot = outp.tile([P, tsz], f32, tag="ot")
                        nc.scalar.activation(
                            ot[:],
                            pss[oi, ti][:],
                            mybir.ActivationFunctionType.Identity,
                            bias=bias_sb[:, o : o + 1],
                            scale=sc_sb[:, 0:1],
                        )
                        nc.sync.dma_start(
                            yt[o * P : (o + 1) * P, t0 : t0 + tsz], ot[:]
                        )
    nc.finalize()
    return nc


def _trunc_ratio(a: np.ndarray) -> float:
    """mean(|trunc_bf16(a)|) / mean(|a|) — the systematic shrink from
    reading only the high 16 bits of each fp32."""
    t = (a.view(np.uint32) & np.uint32(0xFFFF0000)).view(np.float32)
    denom = float(np.abs(a).sum())
    if denom == 0.0:
        return 1.0
    return float(np.abs(t).sum()) / denom


def _prepare(inputs, weight, bias, group_sizes):
    """Build (or reuse) the program and the per-core input maps."""
    inputs = np.ascontiguousarray(np.asarray(inputs, dtype=np.float32))
    weight = np.ascontiguousarray(np.asarray(weight, dtype=np.float32))
    bias = np.ascontiguousarray(np.asarray(bias, dtype=np.float32))
    g = np.asarray(group_sizes).astype(np.int64)

    t_tokens, n_in = inputs.shape
    n_exp, _, n_out = weight.shape
    assert n_exp == N_CORES, f"expected {N_CORES} experts, got {n_exp}"
    offs = np.concatenate([[0], np.cumsum(g)])
    assert offs[-1] == t_tokens, "group_sizes must sum to token count"

    maxg = max(P, int(-(-int(g.max()) // P)) * P)

    key = (maxg, n_in, n_out)
    if key not in _BUILD_CACHE:
        _BUILD_CACHE[key] = _build_program(maxg, n_in, n_out)
    nc = _BUILD_CACHE[key]

    ob = n_out // P
    bias_host = np.ascontiguousarray(bias.reshape(ob, P).T)  # [P, ob]

    # Compensate the mean truncation shrink of both operands.
    scale = 1.0 / (_trunc_ratio(inputs) * _trunc_ratio(weight))
    sc_host = np.full((P, 1), scale, np.float32)

    in_maps = []
    for e in range(n_exp):
        xe = inputs[offs[e] : offs[e + 1]]  # [g_e, n_in]
        xt_e = np.zeros((n_in, maxg), np.float32)
        xt_e[:, : g[e]] = xe.T
        w_e = np.ascontiguousarray(
            weight[e].reshape(kb := n_in // P, P, ob, P).transpose(2, 1, 0, 3)
        )  # [ob, P(k within block), kb, P(o)]
        in_maps.append(
            {"xt": xt_e, "w": w_e, "bias": bias_host, "sc": sc_host}
        )
    return nc, in_maps, g, offs, (t_tokens, n_out)


def kernel(inputs, weight, bias, group_sizes):
    nc, in_maps, g, offs, (t_tokens, n_out) = _prepare(
        inputs, weight, bias, group_sizes
    )
    res = run_bass_kernel_spmd(nc, in_maps, core_ids=list(range(N_CORES)))

    out = np.empty((t_tokens, n_out), np.float32)
    for e in range(N_CORES):
        if g[e] == 0:
            continue
        yt_e = res.results[e]["yt"]  # [n_out, maxg]
        out[offs[e] : offs[e + 1]] = yt_e[:, : g[e]].T
    return out


# revision 16
# speedup vs baseline: 1.0892x; 1.0154x over previous
"""MoE grouped linear (gmm) kernel for 8 Trainium2 NeuronCores.

Strategy (expert parallel, mirrors the shard_map-over-gmm_sharded source):
  - Tokens arrive pre-sorted by expert; group_sizes[e] tokens belong to
    expert e. Core e gets weight[e] plus expert e's token slice, padded to
    MAXG rows so all 8 cores run one SPMD program. The "all-to-all" routing
    is host-side slicing, since kernel() sees the full inputs.
  - Per core we compute y_e^T = W_e^T @ X_e^T (out^T orientation): the
    weight tiles are the PE's stationary operand in natural [K, O] layout
    and X^T (prepared host-side) streams as the moving operand.
  - fp32 inputs are DMA'd untouched into resident SBUF tiles; the PE reads
    the high half of each fp32 word as bf16 through a bitcast + stride-2
    access pattern (truncation toward zero). The mean truncation shrink is
    measured host-side and compensated via the ScalarE evacuation scale;
    the per-partition bias is fused into the same instruction. PSUM
    accumulates in fp32.
Host then unpads/concatenates per-expert outputs back to [T, Out] fp32.
"""

import numpy as np

import concourse.bass as bass
from concourse import bacc
import concourse.mybir as mybir
import concourse.tile as tile
from concourse.bass_utils import run_bass_kernel_spmd

N_CORES = 8
P = 128

_BUILD_CACHE: dict = {}


def _t_chunks(maxg: int) -> list[tuple[int, int]]:
    """Split the token free-dim into PSUM-bank-sized (<=512) chunks."""
    n = (maxg + 511) // 512
    base = ((maxg // n + P - 1) // P) * P
    chunks = []
    off = 0
    while off < maxg:
        sz = min(base, maxg - off)
        chunks.append((off, sz))
        off += sz
    return chunks


def _build_program(maxg: int, n_in: int, n_out: int):
    kb = n_in // P   # contraction blocks
    ob = n_out // P  # output-row blocks
    f32 = mybir.dt.float32
    bf16 = mybir.dt.bfloat16

    nc = bacc.Bacc(
        "TRN2", target_bir_lowering=False, debug=False, num_devices=N_CORES
    )
    xt = nc.dram_tensor("xt", [n_in, maxg], f32, kind="ExternalInput")
    # W pre-tiled host-side: [ob, P(partition of k-block), kb, P(o)] so each
    # o-slab DMA is fully contiguous per partition (8 KiB segments).
    w = nc.dram_tensor("w", [ob, P, kb, P], f32, kind="ExternalInput")
    bias = nc.dram_tensor("bias", [P, ob], f32, kind="ExternalInput")
    sc = nc.dram_tensor("sc", [P, 1], f32, kind="ExternalInput")
    yt = nc.dram_tensor("yt", [n_out, maxg], f32, kind="ExternalOutput")

    chunks = _t_chunks(maxg)

    GRP = 4  # o-blocks processed concurrently (GRP * len(chunks) PSUM banks)
    XQ = 4   # k-slabs per X quarter-tile

    with tile.TileContext(nc) as tc:
        with (
            tc.tile_pool(name="const", bufs=1) as constp,
            tc.tile_pool(name="xtsb", bufs=1) as xtp,
            tc.tile_pool(name="wsb", bufs=2 * GRP) as wp,
            tc.tile_pool(name="outsb", bufs=2 * GRP) as outp,
            tc.tile_pool(name="psum", bufs=1, space="PSUM") as psump,
        ):
            bias_sb = constp.tile([P, ob], f32)
            nc.sync.dma_start(bias_sb[:], bias[:])
            sc_sb = constp.tile([P, 1], f32)
            nc.sync.dma_start(sc_sb[:], sc[:])

            def load_w(o):
                w_o = wp.tile([P, kb, P], f32, tag="wo", name=f"w{o}")
                nc.sync.dma_start(w_o[:], w[o])
                return w_o

            def load_xq(q):
                xq = xtp.tile([P, XQ, maxg], f32, tag=f"xq{q}", name=f"xq{q}")
                nc.sync.dma_start(
                    xq[:],
                    xt[q * XQ * P : (q + 1) * XQ * P, :].rearrange(
                        "(k p) t -> p k t", p=P
                    ),
                )
                return xq[:].bitcast(bf16).rearrange(
                    "p k (t two) -> p k t two", two=2
                )

            # Interleave the prologue DMAs: the single HW ring delivers in
            # order, and group 0 needs w0..w3 plus all of X before its end.
            w_pref = {}
            xqs = [None] * (kb // XQ)
            for kind, i in (
                ("w", 0), ("x", 0), ("w", 1), ("x", 1),
                ("w", 2), ("w", 3), ("x", 2), ("x", 3),
            ):
                if kind == "w":
                    w_pref[i] = load_w(i)
                else:
                    xqs[i] = load_xq(i)

            def evac(ps, o, t0, tsz, engine):
                """PSUM -> SBUF with fused scale + per-o bias, then store."""
                ot = outp.tile([P, tsz], f32, tag="ot", name=f"ot{o}_{t0}")
                if engine == 0:
                    nc.scalar.activation(
                        ot[:],
                        ps[:],
                        mybir.ActivationFunctionType.Identity,
                        bias=bias_sb[:, o : o + 1],
                        scale=sc_sb[:, 0:1],
                    )
                else:
                    nc.vector.tensor_scalar(
                        ot[:],
                        ps[:],
                        sc_sb[:, 0:1],
                        bias_sb[:, o : o + 1],
                        mybir.AluOpType.mult,
                        mybir.AluOpType.add,
                    )
                nc.sync.dma_start(yt[o * P : (o + 1) * P, t0 : t0 + tsz], ot[:])

            # Group 0 (o-blocks 0..GRP-1) runs k-major so every arriving
            # X-slab immediately feeds GRP o-columns of PE work; it owns all
            # 2*GRP PSUM banks.
            g0 = list(range(GRP))
            wovs0 = [
                w_pref.pop(o)[:]
                .bitcast(bf16)
                .rearrange("p k (o two) -> p k o two", two=2)
                for o in g0
            ]
            pss0 = {
                (oi, ti): psump.tile(
                    [P, tsz], f32, tag=f"ps{oi}_{ti}", name=f"ps{oi}_{ti}"
                )
                for oi in g0
                for ti, (t0, tsz) in enumerate(chunks)
            }
            for k in range(kb):
                xvk = xqs[k // XQ]
                for oi in g0:
                    for ti, (t0, tsz) in enumerate(chunks):
                        nc.tensor.matmul(
                            pss0[oi, ti][:],
                            wovs0[oi][:, k, :, 1],
                            xvk[:, k % XQ, t0 : t0 + tsz, 1],
                            start=(k == 0),
                            stop=(k == kb - 1),
                        )
            for oi in g0:
                for ti, (t0, tsz) in enumerate(chunks):
                    evac(pss0[oi, ti], oi, t0, tsz, (oi + ti) % 2)

            # Remaining o-blocks run one at a time: per-bank k-runs rotate
            # through the PSUM banks (released by group 0 in the same
            # order), and evacuations pipeline under the next bank's MMs.
            for o in range(GRP, ob):
                w_o = w_pref.pop(o) if o in w_pref else load_w(o)
                wov = (
                    w_o[:]
                    .bitcast(bf16)
                    .rearrange("p k (o two) -> p k o two", two=2)
                )
                for ti, (t0, tsz) in enumerate(chunks):
                    ps = psump.tile(
                        [P, tsz],
                        f32,
                        tag=f"ps{o % GRP}_{ti}",
                        name=f"ps{o}_{ti}",
                    )
                    for k in range(kb):
                        nc.tensor.matmul(
                            ps[:],
                            wov[:, k, :, 1],
                            xqs[k // XQ][:, k % XQ, t0 : t0 + tsz, 1],
                            start=(k == 0),
                            stop=(k == kb - 1),
                        )
                    evac(ps, o, t0, tsz, (o + ti) % 2)
    nc.finalize()
    return nc


def _trunc_ratio(a: np.ndarray) -> float:
    """mean(|trunc_bf16(a)|) / mean(|a|) — the systematic shrink from
    reading only the high 16 bits of each fp32."""
    t = (a.view(np.uint32) & np.uint32(0xFFFF0000)).view(np.float32)
    denom = float(np.abs(a).sum())
    if denom == 0.0:
        return 1.0
    return float(np.abs(t).sum()) / denom


def _prepare(inputs, weight, bias, group_sizes):
    """Build (or reuse) the program and the per-core input maps."""
    inputs = np.ascontiguousarray(np.asarray(inputs, dtype=np.float32))
    weight = np.ascontiguousarray(np.asarray(weight, dtype=np.float32))
    bias = np.ascontiguousarray(np.asarray(bias, dtype=np.float32))
    g = np.asarray(group_sizes).astype(np.int64)

    t_tokens, n_in = inputs.shape
    n_exp, _, n_out = weight.shape
    assert n_exp == N_CORES, f"expected {N_CORES} experts, got {n_exp}"
    offs = np.concatenate([[0], np.cumsum(g)])
    assert offs[-1] == t_tokens, "group_sizes must sum to token count"

    maxg = max(P, int(-(-int(g.max()) // P)) * P)

    key = (maxg, n_in, n_out)
    if key not in _BUILD_CACHE:
        _BUILD_CACHE[key] = _build_program(maxg, n_in, n_out)
    nc = _BUILD_CACHE[key]

    ob = n_out // P
    bias_host = np.ascontiguousarray(bias.reshape(ob, P).T)  # [P, ob]

    # Compensate the mean truncation shrink of both operands.
    scale = 1.0 / (_trunc_ratio(inputs) * _trunc_ratio(weight))
    sc_host = np.full((P, 1), scale, np.float32)

    in_maps = []
    for e in range(n_exp):
        xe = inputs[offs[e] : offs[e + 1]]  # [g_e, n_in]
        xt_e = np.zeros((n_in, maxg), np.float32)
        xt_e[:, : g[e]] = xe.T
        w_e = np.ascontiguousarray(
            weight[e].reshape(kb := n_in // P, P, ob, P).transpose(2, 1, 0, 3)
        )  # [ob, P(k within block), kb, P(o)]
        in_maps.append(
            {"xt": xt_e, "w": w_e, "bias": bias_host, "sc": sc_host}
        )
    return nc, in_maps, g, offs, (t_tokens, n_out)


def kernel(inputs, weight, bias, group_sizes):
    nc, in_maps, g, offs, (t_tokens, n_out) = _prepare(
        inputs, weight, bias, group_sizes
    )
    res = run_bass_kernel_spmd(nc, in_maps, core_ids=list(range(N_CORES)))

    out = np.empty((t_tokens, n_out), np.float32)
    for e in range(N_CORES):
        if g[e] == 0:
            continue
        yt_e = res.results[e]["yt"]  # [n_out, maxg]
        out[offs[e] : offs[e + 1]] = yt_e[:, : g[e]].T
    return out
